# revision 5
# baseline (speedup 1.0000x reference)
"""Distributed ISTFT kernel for Trainium2 (8 NeuronCores, Bass/Tile).

Math (matches the jax reference):
  z: [2, 513, T] one-sided spectrum (real/imag), T = 8192 frames.
  Hermitian extension + ifft(1024) + window + overlap-add (hop 256) +
  divide by overlapped window sum + trim 512 each side -> [2, 2096896].

cos/sin-split formulation (half the PE work of the direct method):
  fr[n]      = c[n] + s[n]          (n = 0..511)
  fr[1024-n] = c[n] - s[n]
  with c = Cw^T @ zr[0:512], s = Sw^T @ zi[1:512] and the window plus
  the interior 1/win_sum (= 0.5) folded into Cw/Sw.  Each (c, s) pair
  serves two output samples, so the 1024-deep contraction of the direct
  method becomes two 512-deep ones -> 2x fewer PE cycles.

  Output block b (256 samples, trim folded in):
    O[256b + r] = fr_{b+2}[r] + fr_{b+1}[256+r] + fr_b[512+r] + fr_{b-1}[768+r]
  The q=0,1 terms read fr_plus ascending (vector adds with column
  shifts).  The q=2,3 terms read fr_minus at REVERSED sample index;
  partition reversal is illegal on the vector engines, so those terms
  are assembled on the PE with an anti-diagonal permutation stationary
  (J1) accumulating into a PSUM tile, then fused with the q=0,1 adds.

  Rank-1 leftovers (zr[512] row, the n=512 column "u", and the entire
  rank-2 imaginary channel) are added on the host in f32 - they are
  fixed linear functionals of single input rows.

  Frame axis sharded: core c owns output blocks 1024c..1024c+1023 and
  loads frame slots sigma = 0..1026 (global frame f = 1024c - 1 + sigma,
  zero-padded outside [0, 8192)).  No cross-core communication.
"""

import numpy as np
import ml_dtypes

N_FFT = 1024
HOP = 256
T_FRAMES = 8192
N_CORES = 8
SLOTS = 1027          # frame slots per core
NB = 1024             # output blocks per core (core 7 uses 1023)
NBT = 8191            # total output blocks

BF16 = ml_dtypes.bfloat16

# sigma spans for the c/s matmul sweeps and beta spans for O assembly
SSPANS = [(0, 384), (384, 384), (768, 259)]
OSPANS = [(0, 512), (512, 512)]

_CACHE = {}


def _consts():
    if "cw" in _CACHE:
        return
    N = N_FFT
    w = 0.5 * (1.0 - np.cos(2.0 * np.pi * np.arange(N) / N))  # periodic Hann
    g = np.full(513, 2.0)
    g[0] = 1.0
    g[512] = 1.0
    k = np.arange(512)[:, None]
    n = np.arange(512)[None, :]
    cw = (g[:512, None] / N) * np.cos(2 * np.pi * k * n / N) * w[None, :512] * 0.5
    kS = np.arange(1, 512)[:, None]
    sw = np.zeros((512, 512))
    sw[:511] = (-2.0 / N) * np.sin(2 * np.pi * kS * n / N) * w[None, :512] * 0.5
    j1 = np.zeros((128, 128))
    m = np.arange(1, 128)
    j1[128 - m, m] = 1.0
    _CACHE["w"] = w
    _CACHE["cw"] = np.ascontiguousarray(cw.astype(BF16))
    _CACHE["sw"] = np.ascontiguousarray(sw.astype(BF16))
    _CACHE["j1"] = np.ascontiguousarray(j1.astype(BF16))


def _build_nc():
    from contextlib import ExitStack

    import concourse.tile as tile
    from concourse import bacc, mybir

    f32 = mybir.dt.float32
    bf = mybir.dt.bfloat16
    ADD = mybir.AluOpType.add
    SUB = mybir.AluOpType.subtract

    nc = bacc.Bacc("TRN2", target_bir_lowering=False, debug=False,
                   num_devices=N_CORES)

    zr_d = nc.dram_tensor("zr", [512, SLOTS], bf, kind="ExternalInput")
    zi_d = nc.dram_tensor("zi", [512, SLOTS], bf, kind="ExternalInput")
    cw_d = nc.dram_tensor("cw", [512, 512], bf, kind="ExternalInput")
    sw_d = nc.dram_tensor("sw", [512, 512], bf, kind="ExternalInput")
    j1_d = nc.dram_tensor("j1", [128, 128], bf, kind="ExternalInput")
    o_d = nc.dram_tensor("out", [256, NB], bf, kind="ExternalOutput")

    with tile.TileContext(nc) as tc, ExitStack() as ctx:
        dat = ctx.enter_context(tc.tile_pool(name="dat", bufs=1))
        frt = ctx.enter_context(tc.tile_pool(name="frt", bufs=1))
        osb = ctx.enter_context(tc.tile_pool(name="osb", bufs=4))
        psp = ctx.enter_context(tc.tile_pool(name="psp", bufs=8, space="PSUM"))

        # ---- input DMA: s-path (zi/sw) first on both queues, then c-path
        ZI, SW, ZR, CW = [], [], [], []
        for a in range(4):
            t = dat.tile([128, SLOTS], bf, tag=f"zi{a}")
            nc.sync.dma_start(out=t[:], in_=zi_d.ap()[128 * a:128 * (a + 1), :])
            ZI.append(t)
            t = dat.tile([128, 512], bf, tag=f"sw{a}")
            nc.scalar.dma_start(out=t[:], in_=sw_d.ap()[128 * a:128 * (a + 1), :])
            SW.append(t)
        for a in range(4):
            t = dat.tile([128, SLOTS], bf, tag=f"zr{a}")
            nc.sync.dma_start(out=t[:], in_=zr_d.ap()[128 * a:128 * (a + 1), :])
            ZR.append(t)
            t = dat.tile([128, 512], bf, tag=f"cw{a}")
            nc.scalar.dma_start(out=t[:], in_=cw_d.ap()[128 * a:128 * (a + 1), :])
            CW.append(t)
        J1 = dat.tile([128, 128], bf, tag="j1")
        nc.gpsimd.dma_start(out=J1[:], in_=j1_d.ap()[:, :])

        # fr_plus / fr_minus / s-staging tiles (bf16, full sigma width)
        SS = [frt.tile([128, SLOTS], bf, tag=f"ss{j}", name=f"ss{j}")
              for j in range(4)]
        FP = [frt.tile([128, SLOTS], bf, tag=f"fp{j}", name=f"fp{j}")
              for j in range(4)]
        FM = [frt.tile([128, SLOTS], bf, tag=f"fm{j}", name=f"fm{j}")
              for j in range(4)]

        # ---- HAM warm-up: dummy matmuls while the first chunks land
        wtile = dat.tile([4, 384], bf, tag="wt")
        nc.vector.memset(wtile[:], 0.0)
        wps = psp.tile([128, 384], f32, tag="ps", name="warm")
        for i in range(16):
            nc.tensor.matmul(wps[:], lhsT=wtile[:, 0:128], rhs=wtile[:],
                             start=(i == 0), stop=(i == 15))

        def combine(j, s0, S):
            sl = slice(s0, s0 + S)
            nc.scalar.copy(SS[j][:, sl], sps[j][:])
            nc.vector.tensor_tensor(FP[j][:, sl], cps[j][:], SS[j][:, sl], ADD)
            nc.vector.tensor_tensor(FM[j][:, sl], cps[j][:], SS[j][:, sl], SUB)

        # ---- span 0: a-outer (DMA-paced), all 8 psums live
        s0, S = SSPANS[0]
        sps = {}
        cps = {}
        for j in range(4):
            sps[j] = psp.tile([128, S], f32, tag="ps", name=f"s{j}_0")
        for j in range(4):
            cps[j] = psp.tile([128, S], f32, tag="ps", name=f"c{j}_0")
        for a in range(4):
            for j in range(4):
                nc.tensor.matmul(sps[j][:], lhsT=SW[a][:, 128 * j:128 * (j + 1)],
                                 rhs=ZI[a][:, s0:s0 + S],
                                 start=(a == 0), stop=(a == 3))
            for j in range(4):
                nc.tensor.matmul(cps[j][:], lhsT=CW[a][:, 128 * j:128 * (j + 1)],
                                 rhs=ZR[a][:, s0:s0 + S],
                                 start=(a == 0), stop=(a == 3))
        for j in range(4):
            combine(j, s0, S)

        def sweep(si):
            s0, S = SSPANS[si]
            for j in range(4):
                sps[j] = psp.tile([128, S], f32, tag="ps", name=f"s{j}_{si}")
                for a in range(4):
                    nc.tensor.matmul(sps[j][:],
                                     lhsT=SW[a][:, 128 * j:128 * (j + 1)],
                                     rhs=ZI[a][:, s0:s0 + S],
                                     start=(a == 0), stop=(a == 3))
                cps[j] = psp.tile([128, S], f32, tag="ps", name=f"c{j}_{si}")
                for a in range(4):
                    nc.tensor.matmul(cps[j][:],
                                     lhsT=CW[a][:, 128 * j:128 * (j + 1)],
                                     rhs=ZR[a][:, s0:s0 + S],
                                     start=(a == 0), stop=(a == 3))
                combine(j, s0, S)

        def o_span(m, queues):
            b0, B = OSPANS[m]
            for half, (fpA, fpB, fmJ1, fmJ2, slivers) in enumerate((
                    (0, 2, 3, 1, ((2, 0),)),
                    (1, 3, 2, 0, ((3, 1), (1, 0))))):
                op = psp.tile([128, B], f32, tag="ps", name=f"op{m}_{half}")
                nc.tensor.matmul(op[:], lhsT=J1[:],
                                 rhs=FM[fmJ1][:, b0 + 1:b0 + 1 + B],
                                 start=True, stop=False)
                nc.tensor.matmul(op[:], lhsT=J1[:],
                                 rhs=FM[fmJ2][:, b0:b0 + B],
                                 start=False, stop=True)
                tmp = osb.tile([128, B], bf, tag="tmp", name=f"tmp{m}_{half}")
                nc.vector.tensor_tensor(tmp[:], op[:],
                                        FP[fpA][:, b0 + 3:b0 + 3 + B], ADD)
                ot = osb.tile([128, B], bf, tag="ot", name=f"ot{m}_{half}")
                nc.vector.tensor_tensor(ot[:], tmp[:],
                                        FP[fpB][:, b0 + 2:b0 + 2 + B], ADD)
                for (fj, sh) in slivers:
                    nc.gpsimd.tensor_tensor(ot[0:1, :], ot[0:1, :],
                                            FM[fj][0:1, b0 + sh:b0 + sh + B],
                                            ADD)
                queues[half].dma_start(
                    out=o_d.ap()[128 * half:128 * (half + 1), b0:b0 + B],
                    in_=ot[:])

        sweep(1)
        o_span(0, (nc.sync, nc.scalar))
        sweep(2)
        o_span(1, (nc.sync, nc.scalar))

    nc.compile()
    return nc


def _inputs_for_cores(z: np.ndarray, window: np.ndarray):
    _consts()
    in_maps = []
    for c in range(N_CORES):
        G = 1024 * c - 1  # global frame of slot 0
        lo, hi = max(0, G), min(T_FRAMES, G + SLOTS)
        s0, s1 = lo - G, hi - G
        zr_blk = np.zeros((512, SLOTS), np.float32)
        zr_blk[:, s0:s1] = z[0, :512, lo:hi]
        zi_blk = np.zeros((512, SLOTS), np.float32)
        zi_blk[:511, s0:s1] = z[1, 1:512, lo:hi]
        in_maps.append({
            "zr": zr_blk.astype(BF16),
            "zi": zi_blk.astype(BF16),
            "cw": _CACHE["cw"],
            "sw": _CACHE["sw"],
            "j1": _CACHE["j1"],
        })
    return in_maps


def _host_post(z: np.ndarray, window: np.ndarray, dev0: np.ndarray):
    """dev0: [256, NBT] f32 device ch0 (r, global block). Returns [2, L] f32."""
    N = N_FFT
    w = window.astype(np.float64)
    zr, zi = z[0].astype(np.float64), z[1].astype(np.float64)
    b = np.arange(NBT)

    out0 = dev0.astype(np.float64)
    # u-row: fr[512] = c[512] over k=0..511 (k=512 lives in the zr512 term)
    g = np.full(512, 2.0)
    g[0] = 1.0
    u = (0.5 / N) * ((g * ((-1.0) ** np.arange(512))) @ zr[:512])
    u_p = np.zeros(T_FRAMES + 4)
    u_p[1:T_FRAMES + 1] = u
    out0[0] += u_p[b + 1]
    # zr[512] row: O += sum_q Cw512[256q+r] * zr512[b+2-q]
    cw512 = (1.0 / N) * np.cos(np.pi * np.arange(N)) * w * 0.5
    zr512_p = np.zeros(T_FRAMES + 4)
    zr512_p[1:T_FRAMES + 1] = zr[512]
    for q in range(4):
        out0 += cw512[256 * q:256 * q + 256, None] * zr512_p[None, b + 3 - q]
    # ch1 (imag): rank-2 in zi[0], zi[512]
    wI = w * (0.5 / N)
    zi0_p = np.zeros(T_FRAMES + 4)
    zi0_p[1:T_FRAMES + 1] = zi[0]
    zi512_p = np.zeros(T_FRAMES + 4)
    zi512_p[1:T_FRAMES + 1] = zi[512]
    alt = (-1.0) ** np.arange(256)
    out1 = np.zeros((256, NBT))
    for q in range(4):
        out1 += wI[256 * q:256 * q + 256, None] * (
            zi0_p[None, b + 3 - q] + alt[:, None] * zi512_p[None, b + 3 - q])

    out = np.stack([out0.T.reshape(-1), out1.T.reshape(-1)])
    out[:, :256] *= 2.0 / (2.0 - w[768:1024])
    out[:, -256:] *= 2.0 / (2.0 - w[0:256])
    return np.ascontiguousarray(out.astype(np.float32))


def kernel(z: np.ndarray, window: np.ndarray) -> np.ndarray:
    from concourse.bass_utils import run_bass_kernel_spmd

    z = np.asarray(z, dtype=np.float32)
    window = np.asarray(window, dtype=np.float32)

    nc = _CACHE.get("nc")
    if nc is None:
        nc = _build_nc()
        _CACHE["nc"] = nc

    in_maps = _inputs_for_cores(z, window)
    res = run_bass_kernel_spmd(nc, in_maps, list(range(N_CORES)))

    parts = []
    for c in range(N_CORES):
        nb = NB if c < N_CORES - 1 else NB - 1
        o = res.results[c]["out"]  # [256, NB] bf16
        parts.append(np.asarray(o)[:, :nb].astype(np.float32))
    dev0 = np.concatenate(parts, axis=1)  # [256, NBT]
    return _host_post(z, window, dev0)


# revision 8
# speedup vs baseline: 1.0647x; 1.0647x over previous
"""Distributed ISTFT kernel for Trainium2 (8 NeuronCores, Bass/Tile).

Math (matches the jax reference):
  z: [2, 513, T] one-sided spectrum (real/imag), T = 8192 frames.
  Hermitian extension + ifft(1024) + window + overlap-add (hop 256) +
  divide by overlapped window sum + trim 512 each side -> [2, 2096896].

cos/sin-split formulation (half the PE work of the direct method):
  fr[n]      = c[n] + s[n]          (n = 0..511)
  fr[1024-n] = c[n] - s[n]
  with c = Cw^T @ zr[0:512], s = Sw^T @ zi[1:512] and the window plus
  the interior 1/win_sum (= 0.5) folded into Cw/Sw.  Each (c, s) pair
  serves two output samples, so the 1024-deep contraction of the direct
  method becomes two 512-deep ones -> 2x fewer PE cycles.

  Output block b (256 samples, trim folded in):
    O[256b + r] = fr_{b+2}[r] + fr_{b+1}[256+r] + fr_b[512+r] + fr_{b-1}[768+r]
  q=0,1 read fr_plus ascending (DVE adds with column shifts).  q=2,3
  read fr_minus at REVERSED sample index; partition reversal is illegal
  on the vector engines, so those terms are assembled on the PE with an
  anti-diagonal permutation stationary (J1) accumulating into PSUM,
  single-row leftovers ride the same PSUM via one-hot K=1 matmuls.

  Rank-1 leftovers (zr[512] row, the n=512 column "u", and the entire
  rank-2 imaginary channel) are added on the host in f32 - they are
  fixed linear functionals of single input rows.

  Frame axis sharded: core c owns output blocks 1024c..1024c+1023 and
  loads frame slots sigma = 0..1026 (global frame f = 1024c - 1 + sigma,
  zero-padded outside [0, 8192)).  No cross-core communication.
"""

import numpy as np
import ml_dtypes

N_FFT = 1024
HOP = 256
T_FRAMES = 8192
N_CORES = 8
SLOTS = 1027          # frame slots per core
NB = 1024             # output blocks per core (core 7 uses 1023)
NBT = 8191            # total output blocks

BF16 = ml_dtypes.bfloat16

# sigma spans for the c/s matmul sweeps; beta spans for O assembly.
# o-spans 0..2 only need sigma < 771 (covered by sweeps 0-1), so only
# o-span 3 trails the last sweep.
SSPANS = [(0, 384), (384, 387), (771, 256)]
OSPANS = [(0, 256), (256, 256), (512, 256), (768, 256)]

_CACHE = {}


def _consts():
    if "m" in _CACHE:
        return
    N = N_FFT
    w = 0.5 * (1.0 - np.cos(2.0 * np.pi * np.arange(N) / N))  # periodic Hann
    g = np.full(513, 2.0)
    g[0] = 1.0
    g[512] = 1.0
    k = np.arange(512)[:, None]
    n = np.arange(512)[None, :]
    cw = (g[:512, None] / N) * np.cos(2 * np.pi * k * n / N) * w[None, :512] * 0.5
    kS = np.arange(1, 512)[:, None]
    sw = np.zeros((512, 512))
    sw[:511] = (-2.0 / N) * np.sin(2 * np.pi * kS * n / N) * w[None, :512] * 0.5
    # chunk a -> rows 128a..128a+127, cols [cw_a | sw_a]
    m = np.empty((512, 1024))
    for a in range(4):
        m[128 * a:128 * a + 128, 0:512] = cw[128 * a:128 * a + 128]
        m[128 * a:128 * a + 128, 512:1024] = sw[128 * a:128 * a + 128]
    j1 = np.zeros((129, 128))
    mm = np.arange(1, 128)
    j1[128 - mm, mm] = 1.0
    j1[128, 0] = 1.0          # row 128 = one-hot E0 (target partition 0)
    _CACHE["w"] = w
    _CACHE["m"] = np.ascontiguousarray(m.astype(BF16))
    _CACHE["j1"] = np.ascontiguousarray(j1.astype(BF16))


def _build_nc():
    from contextlib import ExitStack

    import concourse.tile as tile
    from concourse import bacc, mybir

    f32 = mybir.dt.float32
    bf = mybir.dt.bfloat16
    ADD = mybir.AluOpType.add
    SUB = mybir.AluOpType.subtract

    nc = bacc.Bacc("TRN2", target_bir_lowering=False, debug=False,
                   num_devices=N_CORES)

    zr_d = nc.dram_tensor("zr", [512, SLOTS], bf, kind="ExternalInput")
    zi_d = nc.dram_tensor("zi", [512, SLOTS], bf, kind="ExternalInput")
    m_d = nc.dram_tensor("m", [512, 1024], bf, kind="ExternalInput")
    j1_d = nc.dram_tensor("j1", [129, 128], bf, kind="ExternalInput")
    o_d = nc.dram_tensor("out", [256, NB], bf, kind="ExternalOutput")

    with tile.TileContext(nc) as tc, ExitStack() as ctx:
        dat = ctx.enter_context(tc.tile_pool(name="dat", bufs=1))
        frt = ctx.enter_context(tc.tile_pool(name="frt", bufs=1))
        osb = ctx.enter_context(tc.tile_pool(name="osb", bufs=2))
        psp = ctx.enter_context(tc.tile_pool(name="psp", bufs=8, space="PSUM"))

        # ---- input DMA on 4 queues, k-chunk-set order (a ascending)
        J1 = dat.tile([128, 128], bf, tag="j1")
        nc.scalar.dma_start(out=J1[:], in_=j1_d.ap()[0:128, :])
        E0 = dat.tile([1, 128], bf, tag="e0")
        nc.scalar.dma_start(out=E0[:], in_=j1_d.ap()[128:129, :])
        ZI, ZR, CW, SW = [None] * 4, [None] * 4, [None] * 4, [None] * 4
        for a in range(4):
            t = dat.tile([128, SLOTS], bf, tag=f"zi{a}", name=f"zi{a}")
            nc.sync.dma_start(out=t[:], in_=zi_d.ap()[128 * a:128 * (a + 1), :])
            ZI[a] = t
            t = dat.tile([128, SLOTS], bf, tag=f"zr{a}", name=f"zr{a}")
            nc.gpsimd.dma_start(out=t[:], in_=zr_d.ap()[128 * a:128 * (a + 1), :])
            ZR[a] = t
            t = dat.tile([128, 1024], bf, tag=f"m{a}", name=f"m{a}")
            nc.scalar.dma_start(out=t[:], in_=m_d.ap()[128 * a:128 * (a + 1), :])
            CW[a] = t[:, 0:512]
            SW[a] = t[:, 512:1024]

        # fr_plus / fr_minus / s-staging tiles (bf16, full sigma width)
        SS = [frt.tile([128, SLOTS], bf, tag=f"ss{j}", name=f"ss{j}")
              for j in range(4)]
        FP = [frt.tile([128, SLOTS], bf, tag=f"fp{j}", name=f"fp{j}")
              for j in range(4)]
        FM = [frt.tile([128, SLOTS], bf, tag=f"fm{j}", name=f"fm{j}")
              for j in range(4)]

        # ---- HAM warm-up: dummy matmuls while the first chunks land
        wtile = dat.tile([4, 384], bf, tag="wt")
        nc.vector.memset(wtile[:], 0.0)
        wps = psp.tile([128, 384], f32, tag="ps", name="warm")
        for i in range(12):
            nc.tensor.matmul(wps[:], lhsT=wtile[:, 0:128], rhs=wtile[:],
                             start=(i == 0), stop=(i == 11))

        sps = {}
        cps = {}

        def combine(j, s0, S):
            sl = slice(s0, s0 + S)
            nc.scalar.copy(SS[j][:, sl], sps[j][:])
            nc.vector.tensor_tensor(FM[j][:, sl], cps[j][:], SS[j][:, sl], SUB)
            nc.vector.tensor_tensor(FP[j][:, sl], cps[j][:], SS[j][:, sl], ADD)

        # ---- span 0: a-outer (DMA-paced), all 8 psums live
        s0, S = SSPANS[0]
        for j in range(4):
            sps[j] = psp.tile([128, S], f32, tag="ps", name=f"s{j}_0")
        for j in range(4):
            cps[j] = psp.tile([128, S], f32, tag="ps", name=f"c{j}_0")
        for a in range(4):
            for j in range(4):
                nc.tensor.matmul(sps[j][:], lhsT=SW[a][:, 128 * j:128 * (j + 1)],
                                 rhs=ZI[a][:, s0:s0 + S],
                                 start=(a == 0), stop=(a == 3))
            for j in range(4):
                nc.tensor.matmul(cps[j][:], lhsT=CW[a][:, 128 * j:128 * (j + 1)],
                                 rhs=ZR[a][:, s0:s0 + S],
                                 start=(a == 0), stop=(a == 3))
        for j in range(4):
            combine(j, s0, S)

        def sweep_group(si, j):
            s0, S = SSPANS[si]
            sps[j] = psp.tile([128, S], f32, tag="ps", name=f"s{j}_{si}")
            for a in range(4):
                nc.tensor.matmul(sps[j][:],
                                 lhsT=SW[a][:, 128 * j:128 * (j + 1)],
                                 rhs=ZI[a][:, s0:s0 + S],
                                 start=(a == 0), stop=(a == 3))
            cps[j] = psp.tile([128, S], f32, tag="ps", name=f"c{j}_{si}")
            for a in range(4):
                nc.tensor.matmul(cps[j][:],
                                 lhsT=CW[a][:, 128 * j:128 * (j + 1)],
                                 rhs=ZR[a][:, s0:s0 + S],
                                 start=(a == 0), stop=(a == 3))
            combine(j, s0, S)

        # O half-span configs: (fpA(+3), fpB(+2), J FM(+1), J FM(+0), slivers)
        OCONF = ((0, 2, 3, 1, ((2, 0),)),
                 (1, 3, 2, 0, ((3, 1), (1, 0))))

        def o_span(m, queues):
            b0, B = OSPANS[m]
            for half, (fpA, fpB, fmJ1, fmJ2, slivers) in enumerate(OCONF):
                op = psp.tile([128, B], f32, tag="ps", name=f"op{m}_{half}")
                nc.tensor.matmul(op[:], lhsT=J1[:],
                                 rhs=FM[fmJ1][:, b0 + 1:b0 + 1 + B],
                                 start=True, stop=False)
                nc.tensor.matmul(op[:], lhsT=J1[:],
                                 rhs=FM[fmJ2][:, b0:b0 + B],
                                 start=False, stop=False)
                for i, (fj, sh) in enumerate(slivers):
                    nc.tensor.matmul(op[:], lhsT=E0[:],
                                     rhs=FM[fj][0:1, b0 + sh:b0 + sh + B],
                                     start=False, stop=(i == len(slivers) - 1))
                tmp = osb.tile([128, B], bf, tag="tmp", name=f"tmp{m}_{half}")
                nc.vector.tensor_tensor(tmp[:], op[:],
                                        FP[fpA][:, b0 + 3:b0 + 3 + B], ADD)
                ot = osb.tile([128, B], bf, tag="ot", name=f"ot{m}_{half}")
                nc.vector.tensor_tensor(ot[:], tmp[:],
                                        FP[fpB][:, b0 + 2:b0 + 2 + B], ADD)
                queues[half].dma_start(
                    out=o_d.ap()[128 * half:128 * (half + 1), b0:b0 + B],
                    in_=ot[:])

        # sweep 1 with o0's J-matmuls tucked after the first two j-groups
        sweep_group(1, 0)
        sweep_group(1, 1)
        o_span(0, (nc.sync, nc.gpsimd))
        sweep_group(1, 2)
        sweep_group(1, 3)
        o_span(1, (nc.scalar, nc.sync))
        o_span(2, (nc.gpsimd, nc.scalar))
        for j in range(4):
            sweep_group(2, j)
        o_span(3, (nc.sync, nc.scalar))

    nc.compile()
    return nc


def _inputs_for_cores(z: np.ndarray, window: np.ndarray):
    _consts()
    in_maps = []
    for c in range(N_CORES):
        G = 1024 * c - 1  # global frame of slot 0
        lo, hi = max(0, G), min(T_FRAMES, G + SLOTS)
        s0, s1 = lo - G, hi - G
        zr_blk = np.zeros((512, SLOTS), np.float32)
        zr_blk[:, s0:s1] = z[0, :512, lo:hi]
        zi_blk = np.zeros((512, SLOTS), np.float32)
        zi_blk[:511, s0:s1] = z[1, 1:512, lo:hi]
        in_maps.append({
            "zr": zr_blk.astype(BF16),
            "zi": zi_blk.astype(BF16),
            "m": _CACHE["m"],
            "j1": _CACHE["j1"],
        })
    return in_maps


def _host_post(z: np.ndarray, window: np.ndarray, dev0: np.ndarray):
    """dev0: [256, NBT] f32 device ch0 (r, global block). Returns [2, L] f32."""
    N = N_FFT
    w = window.astype(np.float64)
    zr, zi = z[0].astype(np.float64), z[1].astype(np.float64)
    b = np.arange(NBT)

    out0 = dev0.astype(np.float64)
    # u-row: fr[512] = c[512] over k=0..511 (k=512 lives in the zr512 term)
    g = np.full(512, 2.0)
    g[0] = 1.0
    u = (0.5 / N) * ((g * ((-1.0) ** np.arange(512))) @ zr[:512])
    u_p = np.zeros(T_FRAMES + 4)
    u_p[1:T_FRAMES + 1] = u
    out0[0] += u_p[b + 1]
    # zr[512] row: O += sum_q Cw512[256q+r] * zr512[b+2-q]
    cw512 = (1.0 / N) * np.cos(np.pi * np.arange(N)) * w * 0.5
    zr512_p = np.zeros(T_FRAMES + 4)
    zr512_p[1:T_FRAMES + 1] = zr[512]
    for q in range(4):
        out0 += cw512[256 * q:256 * q + 256, None] * zr512_p[None, b + 3 - q]
    # ch1 (imag): rank-2 in zi[0], zi[512]
    wI = w * (0.5 / N)
    zi0_p = np.zeros(T_FRAMES + 4)
    zi0_p[1:T_FRAMES + 1] = zi[0]
    zi512_p = np.zeros(T_FRAMES + 4)
    zi512_p[1:T_FRAMES + 1] = zi[512]
    alt = (-1.0) ** np.arange(256)
    out1 = np.zeros((256, NBT))
    for q in range(4):
        out1 += wI[256 * q:256 * q + 256, None] * (
            zi0_p[None, b + 3 - q] + alt[:, None] * zi512_p[None, b + 3 - q])

    out = np.stack([out0.T.reshape(-1), out1.T.reshape(-1)])
    out[:, :256] *= 2.0 / (2.0 - w[768:1024])
    out[:, -256:] *= 2.0 / (2.0 - w[0:256])
    return np.ascontiguousarray(out.astype(np.float32))


def kernel(z: np.ndarray, window: np.ndarray) -> np.ndarray:
    from concourse.bass_utils import run_bass_kernel_spmd

    z = np.asarray(z, dtype=np.float32)
    window = np.asarray(window, dtype=np.float32)

    nc = _CACHE.get("nc")
    if nc is None:
        nc = _build_nc()
        _CACHE["nc"] = nc

    in_maps = _inputs_for_cores(z, window)
    res = run_bass_kernel_spmd(nc, in_maps, list(range(N_CORES)))

    parts = []
    for c in range(N_CORES):
        nb = NB if c < N_CORES - 1 else NB - 1
        o = res.results[c]["out"]  # [256, NB] bf16
        parts.append(np.asarray(o)[:, :nb].astype(np.float32))
    dev0 = np.concatenate(parts, axis=1)  # [256, NBT]
    return _host_post(z, window, dev0)


# revision 10
# speedup vs baseline: 1.1691x; 1.0980x over previous
"""Distributed ISTFT kernel for Trainium2 (8 NeuronCores, Bass/Tile).

Math (matches the jax reference):
  z: [2, 513, T] one-sided spectrum (real/imag), T = 8192 frames.
  Hermitian extension + ifft(1024) + window + overlap-add (hop 256) +
  divide by overlapped window sum + trim 512 each side -> [2, 2096896].

cos/sin-split formulation (half the PE work of the direct method):
  fr[n]      = c[n] + s[n]          (n = 0..511)
  fr[1024-n] = c[n] - s[n]
  with c = Cw^T @ zr[0:512], s = Sw^T @ zi[1:512] and the window plus
  the interior 1/win_sum (= 0.5) folded into Cw/Sw.  Each (c, s) pair
  serves two output samples, so the 1024-deep contraction of the direct
  method becomes two 512-deep ones -> 2x fewer PE cycles.

  Output block b (256 samples, trim folded in):
    O[256b + r] = fr_{b+2}[r] + fr_{b+1}[256+r] + fr_b[512+r] + fr_{b-1}[768+r]
  q=0,1 read fr_plus ascending (DVE adds with column shifts).  q=2,3
  read fr_minus at REVERSED sample index; partition reversal is illegal
  on the vector engines, so those terms are assembled on the PE with an
  anti-diagonal permutation stationary (J1) accumulating into PSUM,
  single-row leftovers ride the same PSUM via one-hot K=1 matmuls.

  Rank-1 leftovers (zr[512] row, the n=512 column "u", and the entire
  rank-2 imaginary channel) are added on the host in f32 - they are
  fixed linear functionals of single input rows.

  Frame axis sharded: core c owns output blocks 1024c..1024c+1023 and
  loads frame slots sigma = 0..1026 (global frame f = 1024c - 1 + sigma,
  zero-padded outside [0, 8192)).  No cross-core communication.
"""

import numpy as np
import ml_dtypes

N_FFT = 1024
HOP = 256
T_FRAMES = 8192
N_CORES = 8
SLOTS = 1027          # frame slots per core
NB = 1024             # output blocks per core (core 7 uses 1023)
NBT = 8191            # total output blocks

BF16 = ml_dtypes.bfloat16

# sigma spans for the c/s matmul sweeps; beta spans for O assembly.
# o-spans 0..2 only need sigma < 771 (covered by sweeps 0-1), so only
# o-span 3 trails the last sweep.
SSPANS = [(0, 384), (384, 387), (771, 256)]
OSPANS = [(0, 256), (256, 256), (512, 256), (768, 256)]

_CACHE = {}


def _consts():
    if "m" in _CACHE:
        return
    N = N_FFT
    w = 0.5 * (1.0 - np.cos(2.0 * np.pi * np.arange(N) / N))  # periodic Hann
    g = np.full(513, 2.0)
    g[0] = 1.0
    g[512] = 1.0
    k = np.arange(512)[:, None]
    n = np.arange(512)[None, :]
    cw = (g[:512, None] / N) * np.cos(2 * np.pi * k * n / N) * w[None, :512] * 0.5
    kS = np.arange(1, 512)[:, None]
    sw = np.zeros((512, 512))
    sw[:511] = (-2.0 / N) * np.sin(2 * np.pi * kS * n / N) * w[None, :512] * 0.5
    # chunk a -> rows 128a..128a+127, cols [cw_a | sw_a]
    m = np.empty((512, 1024))
    for a in range(4):
        m[128 * a:128 * a + 128, 0:512] = cw[128 * a:128 * a + 128]
        m[128 * a:128 * a + 128, 512:1024] = sw[128 * a:128 * a + 128]
    j1 = np.zeros((128, 384))
    mm = np.arange(1, 128)
    j1[128 - mm, mm] = 1.0     # cols 0:128   = J1 (anti-diagonal)
    j1[:, 128:256] = np.eye(128)   # cols 128:256 = identity (q0 pass-through)
    j1[0, 256] = 1.0           # col 256 row 0 = one-hot E0
    _CACHE["w"] = w
    _CACHE["m"] = np.ascontiguousarray(m.astype(BF16))
    _CACHE["j1"] = np.ascontiguousarray(j1.astype(BF16))


def _build_nc():
    from contextlib import ExitStack

    import concourse.tile as tile
    from concourse import bacc, mybir

    f32 = mybir.dt.float32
    bf = mybir.dt.bfloat16
    ADD = mybir.AluOpType.add
    SUB = mybir.AluOpType.subtract
    MULT = mybir.AluOpType.mult

    nc = bacc.Bacc("TRN2", target_bir_lowering=False, debug=False,
                   num_devices=N_CORES)

    zr_d = nc.dram_tensor("zr", [512, SLOTS], bf, kind="ExternalInput")
    zi_d = nc.dram_tensor("zi", [512, SLOTS], bf, kind="ExternalInput")
    m_d = nc.dram_tensor("m", [512, 1024], bf, kind="ExternalInput")
    j1_d = nc.dram_tensor("j1", [128, 384], bf, kind="ExternalInput")
    o_d = nc.dram_tensor("out", [256, NB], bf, kind="ExternalOutput")

    with tile.TileContext(nc) as tc, ExitStack() as ctx:
        dat = ctx.enter_context(tc.tile_pool(name="dat", bufs=1))
        frt = ctx.enter_context(tc.tile_pool(name="frt", bufs=1))
        osb = ctx.enter_context(tc.tile_pool(name="osb", bufs=2))
        psp = ctx.enter_context(tc.tile_pool(name="psp", bufs=8, space="PSUM"))

        # ---- input DMA on 3 queues, k-chunk-set order (a ascending)
        ZI, ZR, CW, SW = [None] * 4, [None] * 4, [None] * 4, [None] * 4
        for a in range(4):
            t = dat.tile([128, SLOTS], bf, tag=f"zi{a}", name=f"zi{a}")
            nc.sync.dma_start(out=t[:], in_=zi_d.ap()[128 * a:128 * (a + 1), :])
            ZI[a] = t
            t = dat.tile([128, SLOTS], bf, tag=f"zr{a}", name=f"zr{a}")
            nc.gpsimd.dma_start(out=t[:], in_=zr_d.ap()[128 * a:128 * (a + 1), :])
            ZR[a] = t
            t = dat.tile([128, 1024], bf, tag=f"m{a}", name=f"m{a}")
            nc.scalar.dma_start(out=t[:], in_=m_d.ap()[128 * a:128 * (a + 1), :])
            CW[a] = t[:, 0:512]
            SW[a] = t[:, 512:1024]
        JEI = dat.tile([128, 384], bf, tag="jei")
        nc.scalar.dma_start(out=JEI[:], in_=j1_d.ap()[:, :])
        J1 = JEI[:, 0:128]
        I128 = JEI[:, 128:256]
        E0 = JEI[0:1, 256:384]

        # fr_plus / fr_minus / s-staging tiles (bf16, full sigma width)
        SS = [frt.tile([128, SLOTS], bf, tag=f"ss{j}", name=f"ss{j}")
              for j in range(4)]
        FP = [frt.tile([128, SLOTS], bf, tag=f"fp{j}", name=f"fp{j}")
              for j in range(4)]
        FM = [frt.tile([128, SLOTS], bf, tag=f"fm{j}", name=f"fm{j}")
              for j in range(4)]

        # ---- HAM warm-up: dummy matmuls while the first chunks land
        wtile = dat.tile([4, 384], bf, tag="wt")
        nc.vector.memset(wtile[:], 0.0)
        wps = psp.tile([128, 384], f32, tag="ps", name="warm")
        for i in range(12):
            nc.tensor.matmul(wps[:], lhsT=wtile[:, 0:128], rhs=wtile[:],
                             start=(i == 0), stop=(i == 11))

        sps = {}
        cps = {}

        def combine(j, s0, S):
            sl = slice(s0, s0 + S)
            nc.scalar.copy(SS[j][:, sl], sps[j][:])
            nc.vector.tensor_tensor(FM[j][:, sl], cps[j][:], SS[j][:, sl], SUB)
            nc.vector.scalar_tensor_tensor(
                out=FP[j][:, sl], in0=SS[j][:, sl], scalar=2.0,
                in1=FM[j][:, sl], op0=MULT, op1=ADD)

        # ---- span 0: a-outer (DMA-paced), all 8 psums live
        s0, S = SSPANS[0]
        for j in range(4):
            sps[j] = psp.tile([128, S], f32, tag="ps", name=f"s{j}_0")
        for j in range(4):
            cps[j] = psp.tile([128, S], f32, tag="ps", name=f"c{j}_0")
        for a in range(4):
            for j in range(4):
                nc.tensor.matmul(sps[j][:], lhsT=SW[a][:, 128 * j:128 * (j + 1)],
                                 rhs=ZI[a][:, s0:s0 + S],
                                 start=(a == 0), stop=(a == 3))
            for j in range(4):
                nc.tensor.matmul(cps[j][:], lhsT=CW[a][:, 128 * j:128 * (j + 1)],
                                 rhs=ZR[a][:, s0:s0 + S],
                                 start=(a == 0), stop=(a == 3))
        for j in range(4):
            combine(j, s0, S)

        def sweep_group(si, j):
            s0, S = SSPANS[si]
            sps[j] = psp.tile([128, S], f32, tag="ps", name=f"s{j}_{si}")
            for a in range(4):
                nc.tensor.matmul(sps[j][:],
                                 lhsT=SW[a][:, 128 * j:128 * (j + 1)],
                                 rhs=ZI[a][:, s0:s0 + S],
                                 start=(a == 0), stop=(a == 3))
            cps[j] = psp.tile([128, S], f32, tag="ps", name=f"c{j}_{si}")
            for a in range(4):
                nc.tensor.matmul(cps[j][:],
                                 lhsT=CW[a][:, 128 * j:128 * (j + 1)],
                                 rhs=ZR[a][:, s0:s0 + S],
                                 start=(a == 0), stop=(a == 3))
            combine(j, s0, S)

        # O half-span configs: (fpA(+3), fpB(+2), J FM(+1), J FM(+0), slivers)
        OCONF = ((0, 2, 3, 1, ((2, 0),)),
                 (1, 3, 2, 0, ((3, 1), (1, 0))))

        def o_span(m, queues):
            b0, B = OSPANS[m]
            for half, (fpA, fpB, fmJ1, fmJ2, slivers) in enumerate(OCONF):
                op = psp.tile([128, B], f32, tag="ps", name=f"op{m}_{half}")
                nc.tensor.matmul(op[:], lhsT=J1[:],
                                 rhs=FM[fmJ1][:, b0 + 1:b0 + 1 + B],
                                 start=True, stop=False)
                nc.tensor.matmul(op[:], lhsT=J1[:],
                                 rhs=FM[fmJ2][:, b0:b0 + B],
                                 start=False, stop=False)
                for (fj, sh) in slivers:
                    nc.tensor.matmul(op[:], lhsT=E0[:],
                                     rhs=FM[fj][0:1, b0 + sh:b0 + sh + B],
                                     start=False, stop=False)
                nc.tensor.matmul(op[:], lhsT=I128[:],
                                 rhs=FP[fpA][:, b0 + 3:b0 + 3 + B],
                                 start=False, stop=True)
                ot = osb.tile([128, B], bf, tag="ot", name=f"ot{m}_{half}")
                nc.vector.tensor_tensor(ot[:], op[:],
                                        FP[fpB][:, b0 + 2:b0 + 2 + B], ADD)
                queues[half].dma_start(
                    out=o_d.ap()[128 * half:128 * (half + 1), b0:b0 + B],
                    in_=ot[:])

        # sweep 1 with o0's J-matmuls tucked after the first two j-groups
        sweep_group(1, 0)
        sweep_group(1, 1)
        o_span(0, (nc.sync, nc.gpsimd))
        sweep_group(1, 2)
        sweep_group(1, 3)
        o_span(1, (nc.scalar, nc.sync))
        o_span(2, (nc.gpsimd, nc.scalar))
        for j in range(4):
            sweep_group(2, j)
        o_span(3, (nc.sync, nc.scalar))

    nc.compile()
    return nc


def _inputs_for_cores(z: np.ndarray, window: np.ndarray):
    _consts()
    in_maps = []
    for c in range(N_CORES):
        G = 1024 * c - 1  # global frame of slot 0
        lo, hi = max(0, G), min(T_FRAMES, G + SLOTS)
        s0, s1 = lo - G, hi - G
        zr_blk = np.zeros((512, SLOTS), np.float32)
        zr_blk[:, s0:s1] = z[0, :512, lo:hi]
        zi_blk = np.zeros((512, SLOTS), np.float32)
        zi_blk[:511, s0:s1] = z[1, 1:512, lo:hi]
        in_maps.append({
            "zr": zr_blk.astype(BF16),
            "zi": zi_blk.astype(BF16),
            "m": _CACHE["m"],
            "j1": _CACHE["j1"],
        })
    return in_maps


def _host_post(z: np.ndarray, window: np.ndarray, dev0: np.ndarray):
    """dev0: [256, NBT] f32 device ch0 (r, global block). Returns [2, L] f32."""
    N = N_FFT
    w = window.astype(np.float64)
    zr, zi = z[0].astype(np.float64), z[1].astype(np.float64)
    b = np.arange(NBT)

    out0 = dev0.astype(np.float64)
    # u-row: fr[512] = c[512] over k=0..511 (k=512 lives in the zr512 term)
    g = np.full(512, 2.0)
    g[0] = 1.0
    u = (0.5 / N) * ((g * ((-1.0) ** np.arange(512))) @ zr[:512])
    u_p = np.zeros(T_FRAMES + 4)
    u_p[1:T_FRAMES + 1] = u
    out0[0] += u_p[b + 1]
    # zr[512] row: O += sum_q Cw512[256q+r] * zr512[b+2-q]
    cw512 = (1.0 / N) * np.cos(np.pi * np.arange(N)) * w * 0.5
    zr512_p = np.zeros(T_FRAMES + 4)
    zr512_p[1:T_FRAMES + 1] = zr[512]
    for q in range(4):
        out0 += cw512[256 * q:256 * q + 256, None] * zr512_p[None, b + 3 - q]
    # ch1 (imag): rank-2 in zi[0], zi[512]
    wI = w * (0.5 / N)
    zi0_p = np.zeros(T_FRAMES + 4)
    zi0_p[1:T_FRAMES + 1] = zi[0]
    zi512_p = np.zeros(T_FRAMES + 4)
    zi512_p[1:T_FRAMES + 1] = zi[512]
    alt = (-1.0) ** np.arange(256)
    out1 = np.zeros((256, NBT))
    for q in range(4):
        out1 += wI[256 * q:256 * q + 256, None] * (
            zi0_p[None, b + 3 - q] + alt[:, None] * zi512_p[None, b + 3 - q])

    out = np.stack([out0.T.reshape(-1), out1.T.reshape(-1)])
    out[:, :256] *= 2.0 / (2.0 - w[768:1024])
    out[:, -256:] *= 2.0 / (2.0 - w[0:256])
    return np.ascontiguousarray(out.astype(np.float32))


def kernel(z: np.ndarray, window: np.ndarray) -> np.ndarray:
    from concourse.bass_utils import run_bass_kernel_spmd

    z = np.asarray(z, dtype=np.float32)
    window = np.asarray(window, dtype=np.float32)

    nc = _CACHE.get("nc")
    if nc is None:
        nc = _build_nc()
        _CACHE["nc"] = nc

    in_maps = _inputs_for_cores(z, window)
    res = run_bass_kernel_spmd(nc, in_maps, list(range(N_CORES)))

    parts = []
    for c in range(N_CORES):
        nb = NB if c < N_CORES - 1 else NB - 1
        o = res.results[c]["out"]  # [256, NB] bf16
        parts.append(np.asarray(o)[:, :nb].astype(np.float32))
    dev0 = np.concatenate(parts, axis=1)  # [256, NBT]
    return _host_post(z, window, dev0)
